# revision 30
# baseline (speedup 1.0000x reference)
"""Trainium2 Bass kernel for the scatter_memory recurrent MemoryBlock problem.

Reference computation (per batch b):
    qid    = (x - 1) % K + 1
    q      = question_emb[qid]                       # [T, EK]
    inter  = tanh(interaction_emb[x])                # [T, EI]
    w      = softmax(q @ key_memory.T)               # [T, C]
    out[t] = value_memory_init + sum_{s<=t} w[s] (x) inter[s]   # [T, C, EI]

Key algebraic restructuring: every per-token quantity depends only on the
token id x[t] in [0, 220].  So the rank-1 update for token value v is
tabulated once:  UTable[v] = softmax(QG[v] @ keyT) (x) tanh(E[v]),
a [221, 4000] table, and

    out[t] = init + sum_v Counts[t, v] * UTable[v]

where Counts[t, v] = |{s <= t : x[s] = v}| is a cumulative one-hot count.
The count matrix is pure index plumbing on the int32 token stream, so the
host precomputes it and DMAs it in; the device does the real math:
softmax/tanh table build + the big count x table matmuls.  The per-batch
init vector rides as 4 extra contraction rows with host-pinned one counts.

PE scheme: fp8e4m3 DoubleRow matmuls.  DoubleRow contracts 2x113 = 226
rows (full 221-token vocab + 4 init rows + pad) in ONE pass at 0.5
cycles/column.  Counts are small integers (max ~10 for this data, host-
verified <= 15) so they are exact in fp8e4m3; the UTable is split into
fp8 hi + lo planes (U = hi + lo, residual ~0.3%), giving 2 matmuls per
output chunk = 1 PE cycle/column: 64k output cols ~= 27us @2.4GHz.

Precision: fp8 hi/lo table, fp32 PSUM accumulate, fp16 output (host
upcasts).  Measured end-to-end error ~2.6e-3 vs the fp32 reference,
against the 2e-2 harness gate.

Sharding: data-parallel over batch. 32 batches / 8 cores = 4 per core.
With the PE off the critical path, the bound is the mandatory PSUM->SBUF
drain copies (DVE+ACT, DMA cannot read PSUM on TRN2) and the fp16 output
DMA (SP/Pool): all four engines land ~35-40us.  The main loop is
chunk-major with chunk q+1's table-build ops interleaved into chunk q's
unit stream on Pool/DVE so the copy engines never head-of-line block.
"""

import numpy as np

# Problem constants (hardcoded per harness contract).
B, T = 32, 512
K = 110
C = 20
EK = 100
EI = 200
V = 2 * K + 1          # 221 token vocabulary
F = C * EI             # 4000 flattened (C, EI)
NCORES = 8
BPC = B // NCORES      # batches per core = 4
PB = 128               # timesteps per block (partition dim)
NBLK = T // PB         # blocks per batch = 4
VS = 113               # DoubleRow plane split: plane0 = tokens 0..112
P1T = V - VS           # 108 tokens in plane 1 (113..220)
NQ = 4                 # output column chunks
CQ = F // NQ           # 1000 logical cols per chunk
CP = 1024              # padded chunk width in PSUM/stage/DRAM (2 banks)
NU = BPC * NBLK        # 16 (block units per chunk)

_CACHE = {}

# tuning knobs (see sweep.py)
CFG = {
    "copy_pat": "ADADADADADA",  # per-unit copy engine rotation (A=ACT, D=DVE)
    "dma_build": "SSP",         # out-DMA rotation during build windows (q<3)
    "dma_last": "SPS",          # out-DMA rotation in the final window
    "fillers": 2,
    "defer_every": 2,           # emit one deferred build op every N units
    "stage_bufs": 14,
    "tail_units": 1,            # final units with split copy/DMA drain
}


def _build_program():
    import concourse.bass as bass
    import concourse.tile as tile
    from concourse import bacc, mybir

    f32 = mybir.dt.float32
    f16 = mybir.dt.float16
    f8 = mybir.dt.float8e4
    AF = mybir.ActivationFunctionType
    OP = mybir.AluOpType
    DR = mybir.MatmulPerfMode.DoubleRow

    nc = bacc.Bacc("TRN2")

    # ---- DRAM parameters ---------------------------------------------------
    # qkcat = qgt [100,221] | keyt [100,20]                          (f32)
    d_qkcat = nc.dram_tensor("qkcat", [EK, V + C], f32, kind="ExternalInput")
    d_inter = nc.dram_tensor("interemb", [V, EI], f32, kind="ExternalInput")
    # host-precomputed cumulative counts: [113, plane, batch-major tau], fp8
    d_cts = nc.dram_tensor("cts8", [VS, 2, BPC * T], f8, kind="ExternalInput")
    # per-batch init rows, fp8 hi/lo split; row BPC is zeros (pads plane 1)
    d_ihi = nc.dram_tensor("inithi", [BPC + 1, F], f8, kind="ExternalInput")
    d_ilo = nc.dram_tensor("initlo", [BPC + 1, F], f8, kind="ExternalInput")
    d_out = nc.dram_tensor("out", [BPC * T, F], f16, kind="ExternalOutput")

    with tile.TileContext(nc) as tc:
        with (
            tc.tile_pool(name="const", bufs=1) as constp,
            tc.tile_pool(name="ut", bufs=1) as utp,
            tc.tile_pool(name="stagep", bufs=CFG["stage_bufs"]) as stagep,
            tc.tile_pool(name="bigps", bufs=4, space=bass.MemorySpace.PSUM) as bigpsp,
        ):
            # ---- warm the ACT table (1.3us load) under the const DMAs -----
            warm = constp.tile([1, 1], f32)
            nc.gpsimd.memset(warm[:], 0.0)
            nc.scalar.activation(warm[:], warm[:], AF.Exp)

            # ---- load constants -------------------------------------------
            qkcat = constp.tile([EK, V + C], f32)
            nc.sync.dma_start(qkcat[:], d_qkcat[:])
            qgt = qkcat[:, 0:V]
            keyt = qkcat[:, V : V + C]

            in1 = constp.tile([VS, EI], f32)
            nc.sync.dma_start(in1[:], d_inter[0:VS, :])
            in2 = constp.tile([P1T, EI], f32)
            nc.sync.dma_start(in2[:], d_inter[VS:V, :])

            cts = constp.tile([VS, 2, BPC * T], f8)
            nc.gpsimd.dma_start(cts[:], d_cts[:])

            # ---- per-vocab softmax weights (fp32, tiny) -------------------
            # logits live in one recycled bigps slot (both halves bank-
            # aligned) so all 8 PSUM banks go to the pb pipeline
            lgt = bigpsp.tile([PB, CP], f32, name="pb", tag="pb")
            lg1 = lgt[0:VS, 0:C]
            nc.tensor.matmul(lg1, qgt[:, 0:VS], keyt[:], start=True, stop=True,
                             skip_group_check=True)
            lg2 = lgt[0:P1T, 512 : 512 + C]
            nc.tensor.matmul(lg2, qgt[:, VS:V], keyt[:], start=True, stop=True,
                             skip_group_check=True)

            # softmax without max-subtraction: |logits| <= ~45 here, far
            # inside the fp32 exp range, and exp(l)/sum(exp(l)) is exact.
            w1 = constp.tile([VS, C], f32)
            w2 = constp.tile([P1T, C], f32)
            for lg, w, p in ((lg1, w1, VS), (lg2, w2, P1T)):
                sm = constp.tile([p, 1], f32, tag=f"sm{p}")
                nc.scalar.activation(w[:], lg, AF.Exp, accum_out=sm[:])
                rc = constp.tile([p, 1], f32, tag=f"rc{p}")
                nc.vector.reciprocal(rc[:], sm[:])
                nc.vector.tensor_scalar_mul(w[:], w[:], rc[:, 0:1])

            # ---- tanh of interaction embeddings ---------------------------
            t1 = constp.tile([VS, EI], f32)
            nc.scalar.activation(t1[:], in1[:], AF.Tanh)
            t2 = constp.tile([P1T, EI], f32)
            nc.scalar.activation(t2[:], in2[:], AF.Tanh)

            # ---- UTable fp8 hi/lo, one [113, 2, 1000] pair per chunk ------
            # plane 0 rows: tokens 0..112; plane 1 rows 0:108 tokens
            # 113..220, rows 108:112 init batches 0..3, row 112 zero pad
            # (the init DMA covers it so stale fp8 garbage never multiplies).
            uth = [utp.tile([VS, 2, CQ], f8, name=f"uth{q}") for q in range(NQ)]
            utl = [utp.tile([VS, 2, CQ], f8, name=f"utl{q}") for q in range(NQ)]
            for q in range(NQ):
                qs = slice(q * CQ, (q + 1) * CQ)
                nc.sync.dma_start(uth[q][P1T : VS, 1, :], d_ihi[:, qs])
                nc.sync.dma_start(utl[q][P1T : VS, 1, :], d_ilo[:, qs])

            # spin the PE between the logits matmuls and the first block
            # matmul so the p-state ramp completes during the table build
            # (throwaway outputs into recycled bigps slots; the PE clock
            # ramps only while the engine stays continuously busy)
            for _ in range(CFG["fillers"]):
                fill = bigpsp.tile([PB, CP], f32, name="pb", tag="pb")
                nc.tensor.matmul(
                    fill[0:EK, 0 : V + C], qkcat[:, 0:EK], qkcat[:],
                    start=True, stop=True,
                )

            # chunk-0 table build, spread so the PE can start ASAP:
            #   hi plane0 -> ACT, hi plane1 -> DVE, lo plane0 -> DVE
            #   lo plane1 -> Pool (2-op: walrus lowers the fused
            #   scalar_tensor_tensor only on DVE)
            tmpp = constp.tile([P1T, EI], f32, name="tmpp")
            tmpw = [
                constp.tile([VS, CQ], f32, name="tmpw0"),
                constp.tile([P1T, CQ], f32, name="tmpw1"),
            ]

            def wide_aps(plane, q):
                """Broadcast APs for a whole 1000-col chunk of one plane:
                tanh repeated 5x along concepts (stride-0 outer dim), w
                expanded 200x per concept (stride-0 inner dim)."""
                tt = (t1 if plane == 0 else t2)[:]
                ww = (w1 if plane == 0 else w2)[:, 5 * q : 5 * q + 5]
                t_rep = bass.AP(
                    tt.tensor, tt.offset, [tt.ap[0], [0, 5], [1, EI]]
                )
                w_rep = bass.AP(
                    ww.tensor, ww.offset, [ww.ap[0], [ww.ap[1][0], 5], [0, EI]]
                )
                return t_rep, w_rep

            def build_ops(q):
                """Yield thunks emitting chunk q's table build: one wide
                mult per (plane, hi) + wide tmp/subtract for lo, all Pool."""
                for plane in range(2):
                    rows = slice(0, VS if plane == 0 else P1T)

                    def hi_op(plane=plane, rows=rows):
                        t_rep, w_rep = wide_aps(plane, q)
                        nc.gpsimd.tensor_tensor(
                            uth[q][rows, plane, :], t_rep, w_rep, op=OP.mult
                        )

                    def tmp_op(plane=plane, rows=rows):
                        t_rep, w_rep = wide_aps(plane, q)
                        nc.gpsimd.tensor_tensor(
                            tmpw[plane][:], t_rep, w_rep, op=OP.mult
                        )

                    def lo_op(plane=plane, rows=rows):
                        nc.gpsimd.tensor_tensor(
                            utl[q][rows, plane, :], tmpw[plane][:],
                            uth[q][rows, plane, :], op=OP.subtract,
                        )

                    yield hi_op
                    yield tmp_op
                    yield lo_op

            # chunk 0 built eagerly up front: three wide ops per engine
            # (Pool: hi plane-0, tmp plane-1, lo plane-1 / DVE: tmp plane-0,
            # hi plane-1, lo plane-0), leaving ACT free to start copies.
            t_rep0, w_rep0 = wide_aps(0, 0)
            t_rep1, w_rep1 = wide_aps(1, 0)
            nc.gpsimd.tensor_tensor(uth[0][:, 0, :], t_rep0, w_rep0, op=OP.mult)
            nc.vector.tensor_tensor(tmpw[0][:], t_rep0, w_rep0, op=OP.mult)
            nc.vector.tensor_tensor(uth[0][0:P1T, 1, :], t_rep1, w_rep1, op=OP.mult)
            nc.gpsimd.tensor_tensor(tmpw[1][:], t_rep1, w_rep1, op=OP.mult)
            nc.vector.tensor_tensor(
                utl[0][:, 0, :], tmpw[0][:], uth[0][:, 0, :], op=OP.subtract
            )
            nc.gpsimd.tensor_tensor(
                utl[0][0:P1T, 1, :], tmpw[1][:], uth[0][0:P1T, 1, :],
                op=OP.subtract,
            )

            # ---- main loop: chunk-major, 4 chunks x 16 block units --------
            # out[t, f] = sum_v CTall[v, t] * UTable[v, f]; chunk q+1's
            # build thunks are interleaved into chunk q's unit stream.
            A, D = nc.scalar, nc.vector
            copy_pat = [A if ch == "A" else D for ch in CFG["copy_pat"]]
            for q in range(NQ):
                deferred = list(build_ops(q + 1)) if q + 1 < NQ else []
                for j in range(NU):
                    idx = q * NU + j
                    b, k = divmod(j, NBLK)
                    ks = slice(b * T + k * PB, b * T + (k + 1) * PB)
                    pb_ = bigpsp.tile([PB, CP], f32, name="pb", tag="pb")
                    # 512+488 halves tile the two PSUM banks exactly
                    for hs in (slice(0, 512), slice(512, CQ)):
                        po = pb_[:, hs]
                        nc.tensor.matmul(
                            po, cts[:, :, ks], uth[q][:, :, hs],
                            start=True, stop=False, perf_mode=DR,
                            skip_group_check=True,
                        )
                        nc.tensor.matmul(
                            po, cts[:, :, ks], utl[q][:, :, hs],
                            start=False, stop=True, perf_mode=DR,
                            skip_group_check=True,
                        )
                    stage = stagep.tile([PB, CQ], f16, tag="stage")
                    last = q == NQ - 1 and j >= NU - CFG["tail_units"]
                    ce = copy_pat[idx % len(copy_pat)]
                    if last:
                        # drain tail: split the copy across both engines and
                        # the DMA across both queues so the final units
                        # finish in half the latency
                        nc.scalar.copy(stage[:, 0:512], pb_[:, 0:512])
                        nc.vector.tensor_copy(stage[:, 512:CQ], pb_[:, 512:CQ])
                    elif ce is nc.scalar:
                        ce.copy(stage[:], pb_[:, 0:CQ])
                    else:
                        ce.tensor_copy(stage[:], pb_[:, 0:CQ])
                    # next chunk's table build rides between units
                    if deferred and j % CFG["defer_every"] == 0:
                        deferred.pop(0)()
                    dst = d_out[j * PB : (j + 1) * PB, q * CQ : (q + 1) * CQ]
                    if last:
                        nc.sync.dma_start(
                            d_out[j * PB : (j + 1) * PB, q * CQ : q * CQ + 512],
                            stage[:, 0:512],
                        )
                        nc.gpsimd.dma_start(
                            d_out[
                                j * PB : (j + 1) * PB,
                                q * CQ + 512 : (q + 1) * CQ,
                            ],
                            stage[:, 512:CQ],
                        )
                    else:
                        pat = CFG["dma_build"] if q < NQ - 1 else CFG["dma_last"]
                        if pat[j % len(pat)] == "S":
                            nc.sync.dma_start(dst, stage[:])
                        else:
                            nc.gpsimd.dma_start(dst, stage[:])
                for th in deferred:
                    th()

    nc.compile()
    return nc


def _host_inputs(x, question_emb, interaction_emb, key_memory, value_memory_init):
    """Build the shared constant tensors + per-core shards (all numpy)."""
    import ml_dtypes

    F8 = ml_dtypes.float8_e4m3fn
    x = np.asarray(x).astype(np.int32)
    question_emb = np.asarray(question_emb, dtype=np.float32)
    interaction_emb = np.asarray(interaction_emb, dtype=np.float32)
    key_memory = np.asarray(key_memory, dtype=np.float32)
    value_memory_init = np.asarray(value_memory_init, dtype=np.float32)

    v = np.arange(V, dtype=np.int64)
    qid = (v - 1) % K + 1

    qkcat = np.zeros((EK, V + C), np.float32)
    qkcat[:, :V] = question_emb[qid].T
    qkcat[:, V : V + C] = key_memory.T

    consts = {"qkcat": qkcat, "interemb": interaction_emb}

    in_maps = []
    for core in range(NCORES):
        bs = slice(core * BPC, (core + 1) * BPC)
        xc = x[bs]                                  # [BPC, T]
        # cumulative one-hot counts per batch: ct[v, tau] = #{s<=tau: x[s]=v}
        oh = xc[:, :, None] == np.arange(V)[None, None, :]     # [BPC,T,V]
        ct = np.cumsum(oh, axis=1).transpose(0, 2, 1)          # [BPC,V,T]
        assert ct.max() <= 15, "counts exceed fp8e4m3 exact-integer range"
        cts8 = np.zeros((VS, 2, BPC * T), F8)
        cts8[:, 0, :] = ct[:, 0:VS, :].transpose(1, 0, 2).reshape(VS, BPC * T)
        cts8[0:P1T, 1, :] = (
            ct[:, VS:V, :].transpose(1, 0, 2).reshape(P1T, BPC * T)
        )
        for b in range(BPC):
            cts8[P1T + b, 1, b * T : (b + 1) * T] = 1.0
        initf = value_memory_init[bs].reshape(BPC, F)
        ihi = np.zeros((BPC + 1, F), F8)
        ihi[0:BPC] = initf.astype(F8)
        ilo = np.zeros((BPC + 1, F), F8)
        ilo[0:BPC] = (initf - ihi[0:BPC].astype(np.float32)).astype(F8)
        in_maps.append({**consts, "cts8": cts8, "inithi": ihi, "initlo": ilo})
    return in_maps


def kernel(
    x,
    next_question,
    question_emb,
    interaction_emb,
    key_memory,
    value_memory_init,
):
    from concourse.bass_utils import run_bass_kernel_spmd

    if "nc" not in _CACHE:
        _CACHE["nc"] = _build_program()
    nc = _CACHE["nc"]

    in_maps = _host_inputs(
        x, question_emb, interaction_emb, key_memory, value_memory_init
    )
    res = run_bass_kernel_spmd(nc, in_maps, list(range(NCORES)))
    return _unpack(res.results)


def _unpack(results):
    return np.concatenate(
        [
            np.asarray(r["out"]).astype(np.float32).reshape(BPC, T, C, EI)
            for r in results
        ],
        axis=0,
    )


# revision 31
# speedup vs baseline: 1.0299x; 1.0299x over previous
"""Trainium2 Bass kernel for the scatter_memory recurrent MemoryBlock problem.

Reference computation (per batch b):
    qid    = (x - 1) % K + 1
    q      = question_emb[qid]                       # [T, EK]
    inter  = tanh(interaction_emb[x])                # [T, EI]
    w      = softmax(q @ key_memory.T)               # [T, C]
    out[t] = value_memory_init + sum_{s<=t} w[s] (x) inter[s]   # [T, C, EI]

Key algebraic restructuring: every per-token quantity depends only on the
token id x[t] in [0, 220].  So the rank-1 update for token value v is
tabulated once:  UTable[v] = softmax(QG[v] @ keyT) (x) tanh(E[v]),
a [221, 4000] table, and

    out[t] = init + sum_v Counts[t, v] * UTable[v]

where Counts[t, v] = |{s <= t : x[s] = v}| is a cumulative one-hot count.
The count matrix is pure index plumbing on the int32 token stream, so the
host precomputes it and DMAs it in; the device does the real math:
softmax/tanh table build + the big count x table matmuls.  The per-batch
init vector rides as 4 extra contraction rows with host-pinned one counts.

PE scheme: fp8e4m3 DoubleRow matmuls.  DoubleRow contracts 2x113 = 226
rows (full 221-token vocab + 4 init rows + pad) in ONE pass at 0.5
cycles/column.  Counts are small integers (max ~10 for this data, host-
verified <= 15) so they are exact in fp8e4m3; the UTable is split into
fp8 hi + lo planes (U = hi + lo, residual ~0.3%), giving 2 matmuls per
output chunk = 1 PE cycle/column: 64k output cols ~= 27us @2.4GHz.

Precision: fp8 hi/lo table, fp32 PSUM accumulate, fp16 output (host
upcasts).  Measured end-to-end error ~2.6e-3 vs the fp32 reference,
against the 2e-2 harness gate.

Sharding: data-parallel over batch. 32 batches / 8 cores = 4 per core.
With the PE off the critical path, the bound is the mandatory PSUM->SBUF
drain copies (DVE+ACT, DMA cannot read PSUM on TRN2) and the fp16 output
DMA (SP/Pool): all four engines land ~35-40us.  The main loop is
chunk-major with chunk q+1's table-build ops interleaved into chunk q's
unit stream on Pool/DVE so the copy engines never head-of-line block.
"""

import numpy as np

# Problem constants (hardcoded per harness contract).
B, T = 32, 512
K = 110
C = 20
EK = 100
EI = 200
V = 2 * K + 1          # 221 token vocabulary
F = C * EI             # 4000 flattened (C, EI)
NCORES = 8
BPC = B // NCORES      # batches per core = 4
PB = 128               # timesteps per block (partition dim)
NBLK = T // PB         # blocks per batch = 4
VS = 113               # DoubleRow plane split: plane0 = tokens 0..112
P1T = V - VS           # 108 tokens in plane 1 (113..220)
NQ = 4                 # output column chunks
CQ = F // NQ           # 1000 logical cols per chunk
CP = 1024              # padded chunk width in PSUM/stage/DRAM (2 banks)
NU = BPC * NBLK        # 16 (block units per chunk)

_CACHE = {}

# tuning knobs (see sweep.py)
CFG = {
    "copy_pat": "ADADADADADA",  # per-unit copy engine rotation (A=ACT, D=DVE)
    "dma_build": "SSP",         # out-DMA rotation during build windows (q<3)
    "dma_last": "SPS",          # out-DMA rotation in the final window
    "fillers": 2,
    "defer_every": 2,           # emit one deferred build op every N units
    "stage_bufs": 14,
    "tail_units": 1,            # final units with split copy/DMA drain
}


def _build_program():
    import concourse.bass as bass
    import concourse.tile as tile
    from concourse import bacc, mybir

    f32 = mybir.dt.float32
    f16 = mybir.dt.float16
    f8 = mybir.dt.float8e4
    AF = mybir.ActivationFunctionType
    OP = mybir.AluOpType
    DR = mybir.MatmulPerfMode.DoubleRow

    nc = bacc.Bacc("TRN2")

    # ---- DRAM parameters ---------------------------------------------------
    # qkcat = qgt [100,221] | keyt [100,20]                          (f32)
    d_qkcat = nc.dram_tensor("qkcat", [EK, V + C], f32, kind="ExternalInput")
    d_inter = nc.dram_tensor("interemb", [V, EI], f32, kind="ExternalInput")
    # host-precomputed cumulative counts: [113, plane, batch-major tau], fp8
    d_cts = nc.dram_tensor("cts8", [VS, 2, BPC * T], f8, kind="ExternalInput")
    # per-batch init rows, fp8 hi/lo split; row BPC is zeros (pads plane 1)
    d_ihi = nc.dram_tensor("inithi", [BPC + 1, F], f8, kind="ExternalInput")
    d_ilo = nc.dram_tensor("initlo", [BPC + 1, F], f8, kind="ExternalInput")
    d_out = nc.dram_tensor("out", [BPC * T, F], f16, kind="ExternalOutput")

    with tile.TileContext(nc) as tc:
        with (
            tc.tile_pool(name="const", bufs=1) as constp,
            tc.tile_pool(name="ut", bufs=1) as utp,
            tc.tile_pool(name="stagep", bufs=CFG["stage_bufs"]) as stagep,
            tc.tile_pool(name="bigps", bufs=4, space=bass.MemorySpace.PSUM) as bigpsp,
        ):
            # ---- warm the ACT table (1.3us load) under the const DMAs -----
            warm = constp.tile([1, 1], f32)
            nc.gpsimd.memset(warm[:], 0.0)
            nc.scalar.activation(warm[:], warm[:], AF.Exp)

            # ---- load constants -------------------------------------------
            qkcat = constp.tile([EK, V + C], f32)
            nc.sync.dma_start(qkcat[:], d_qkcat[:])
            qgt = qkcat[:, 0:V]
            keyt = qkcat[:, V : V + C]

            in1 = constp.tile([VS, EI], f32)
            nc.sync.dma_start(in1[:], d_inter[0:VS, :])
            in2 = constp.tile([P1T, EI], f32)
            nc.sync.dma_start(in2[:], d_inter[VS:V, :])

            cts = constp.tile([VS, 2, BPC * T], f8)
            nc.gpsimd.dma_start(cts[:], d_cts[:])

            # ---- per-vocab softmax weights (fp32, tiny) -------------------
            # logits live in one recycled bigps slot (both halves bank-
            # aligned) so all 8 PSUM banks go to the pb pipeline
            lgt = bigpsp.tile([PB, CP], f32, name="pb", tag="pb")
            lg1 = lgt[0:VS, 0:C]
            nc.tensor.matmul(lg1, qgt[:, 0:VS], keyt[:], start=True, stop=True,
                             skip_group_check=True)
            lg2 = lgt[0:P1T, 512 : 512 + C]
            nc.tensor.matmul(lg2, qgt[:, VS:V], keyt[:], start=True, stop=True,
                             skip_group_check=True)

            # softmax without max-subtraction: |logits| <= ~45 here, far
            # inside the fp32 exp range, and exp(l)/sum(exp(l)) is exact.
            w1 = constp.tile([VS, C], f32)
            w2 = constp.tile([P1T, C], f32)
            for lg, w, p in ((lg1, w1, VS), (lg2, w2, P1T)):
                sm = constp.tile([p, 1], f32, tag=f"sm{p}")
                nc.scalar.activation(w[:], lg, AF.Exp, accum_out=sm[:])
                rc = constp.tile([p, 1], f32, tag=f"rc{p}")
                nc.vector.reciprocal(rc[:], sm[:])
                nc.vector.tensor_scalar_mul(w[:], w[:], rc[:, 0:1])

            # ---- tanh of interaction embeddings ---------------------------
            t1 = constp.tile([VS, EI], f32)
            nc.scalar.activation(t1[:], in1[:], AF.Tanh)
            t2 = constp.tile([P1T, EI], f32)
            nc.scalar.activation(t2[:], in2[:], AF.Tanh)

            # ---- UTable fp8 hi/lo, one [113, 2, 1000] pair per chunk ------
            # plane 0 rows: tokens 0..112; plane 1 rows 0:108 tokens
            # 113..220, rows 108:112 init batches 0..3, row 112 zero pad
            # (the init DMA covers it so stale fp8 garbage never multiplies).
            uth = [utp.tile([VS, 2, CQ], f8, name=f"uth{q}") for q in range(NQ)]
            utl = [utp.tile([VS, 2, CQ], f8, name=f"utl{q}") for q in range(NQ)]
            for q in range(NQ):
                qs = slice(q * CQ, (q + 1) * CQ)
                nc.sync.dma_start(uth[q][P1T : VS, 1, :], d_ihi[:, qs])
                nc.sync.dma_start(utl[q][P1T : VS, 1, :], d_ilo[:, qs])

            # spin the PE between the logits matmuls and the first block
            # matmul so the p-state ramp completes during the table build
            # (throwaway outputs into recycled bigps slots; the PE clock
            # ramps only while the engine stays continuously busy)
            for _ in range(CFG["fillers"]):
                fill = bigpsp.tile([PB, CP], f32, name="pb", tag="pb")
                nc.tensor.matmul(
                    fill[0:EK, 0 : V + C], qkcat[:, 0:EK], qkcat[:],
                    start=True, stop=True,
                )

            # chunk-0 table build, spread so the PE can start ASAP:
            #   hi plane0 -> ACT, hi plane1 -> DVE, lo plane0 -> DVE
            #   lo plane1 -> Pool (2-op: walrus lowers the fused
            #   scalar_tensor_tensor only on DVE)
            tmpp = constp.tile([P1T, EI], f32, name="tmpp")
            tmpw = [
                constp.tile([VS, CQ], f32, name="tmpw0"),
                constp.tile([P1T, CQ], f32, name="tmpw1"),
            ]

            def wide_aps(plane, q):
                """Broadcast APs for a whole 1000-col chunk of one plane:
                tanh repeated 5x along concepts (stride-0 outer dim), w
                expanded 200x per concept (stride-0 inner dim)."""
                tt = (t1 if plane == 0 else t2)[:]
                ww = (w1 if plane == 0 else w2)[:, 5 * q : 5 * q + 5]
                t_rep = bass.AP(
                    tt.tensor, tt.offset, [tt.ap[0], [0, 5], [1, EI]]
                )
                w_rep = bass.AP(
                    ww.tensor, ww.offset, [ww.ap[0], [ww.ap[1][0], 5], [0, EI]]
                )
                return t_rep, w_rep

            def build_ops(q):
                """Yield thunks emitting chunk q's table build: one wide
                mult per (plane, hi) + wide tmp/subtract for lo, all Pool."""
                for plane in range(2):
                    rows = slice(0, VS if plane == 0 else P1T)

                    def hi_op(plane=plane, rows=rows):
                        t_rep, w_rep = wide_aps(plane, q)
                        nc.gpsimd.tensor_tensor(
                            uth[q][rows, plane, :], t_rep, w_rep, op=OP.mult
                        )

                    def tmp_op(plane=plane, rows=rows):
                        t_rep, w_rep = wide_aps(plane, q)
                        nc.gpsimd.tensor_tensor(
                            tmpw[plane][:], t_rep, w_rep, op=OP.mult
                        )

                    def lo_op(plane=plane, rows=rows):
                        nc.gpsimd.tensor_tensor(
                            utl[q][rows, plane, :], tmpw[plane][:],
                            uth[q][rows, plane, :], op=OP.subtract,
                        )

                    yield hi_op
                    yield tmp_op
                    yield lo_op

            # chunk 0 built eagerly up front: ACT (idle pre-stream) does the
            # narrow plane-0 hi muls, DVE/Pool the wide remainder.
            for ci in range(5):
                nc.scalar.mul(
                    uth[0][:, 0, ci * EI : (ci + 1) * EI],
                    t1[:], w1[:, ci : ci + 1],
                )
            t_rep0, w_rep0 = wide_aps(0, 0)
            t_rep1, w_rep1 = wide_aps(1, 0)
            nc.vector.tensor_tensor(uth[0][0:P1T, 1, :], t_rep1, w_rep1, op=OP.mult)
            nc.gpsimd.tensor_tensor(tmpw[0][:], t_rep0, w_rep0, op=OP.mult)
            nc.gpsimd.tensor_tensor(tmpw[1][:], t_rep1, w_rep1, op=OP.mult)
            nc.vector.tensor_tensor(
                utl[0][:, 0, :], tmpw[0][:], uth[0][:, 0, :], op=OP.subtract
            )
            nc.gpsimd.tensor_tensor(
                utl[0][0:P1T, 1, :], tmpw[1][:], uth[0][0:P1T, 1, :],
                op=OP.subtract,
            )

            # ---- main loop: chunk-major, 4 chunks x 16 block units --------
            # out[t, f] = sum_v CTall[v, t] * UTable[v, f]; chunk q+1's
            # build thunks are interleaved into chunk q's unit stream.
            A, D = nc.scalar, nc.vector
            copy_pat = [A if ch == "A" else D for ch in CFG["copy_pat"]]
            for q in range(NQ):
                deferred = list(build_ops(q + 1)) if q + 1 < NQ else []
                for j in range(NU):
                    idx = q * NU + j
                    b, k = divmod(j, NBLK)
                    ks = slice(b * T + k * PB, b * T + (k + 1) * PB)
                    pb_ = bigpsp.tile([PB, CP], f32, name="pb", tag="pb")
                    # 512+488 halves tile the two PSUM banks exactly
                    for hs in (slice(0, 512), slice(512, CQ)):
                        po = pb_[:, hs]
                        nc.tensor.matmul(
                            po, cts[:, :, ks], uth[q][:, :, hs],
                            start=True, stop=False, perf_mode=DR,
                            skip_group_check=True,
                        )
                        nc.tensor.matmul(
                            po, cts[:, :, ks], utl[q][:, :, hs],
                            start=False, stop=True, perf_mode=DR,
                            skip_group_check=True,
                        )
                    stage = stagep.tile([PB, CQ], f16, tag="stage")
                    last = q == NQ - 1 and j >= NU - CFG["tail_units"]
                    ce = copy_pat[idx % len(copy_pat)]
                    if last:
                        # drain tail: split the copy across both engines and
                        # the DMA across both queues so the final units
                        # finish in half the latency
                        nc.scalar.copy(stage[:, 0:512], pb_[:, 0:512])
                        nc.vector.tensor_copy(stage[:, 512:CQ], pb_[:, 512:CQ])
                    elif ce is nc.scalar:
                        ce.copy(stage[:], pb_[:, 0:CQ])
                    else:
                        ce.tensor_copy(stage[:], pb_[:, 0:CQ])
                    # next chunk's table build rides between units
                    if deferred and j % CFG["defer_every"] == 0:
                        deferred.pop(0)()
                    dst = d_out[j * PB : (j + 1) * PB, q * CQ : (q + 1) * CQ]
                    if last:
                        nc.sync.dma_start(
                            d_out[j * PB : (j + 1) * PB, q * CQ : q * CQ + 512],
                            stage[:, 0:512],
                        )
                        nc.gpsimd.dma_start(
                            d_out[
                                j * PB : (j + 1) * PB,
                                q * CQ + 512 : (q + 1) * CQ,
                            ],
                            stage[:, 512:CQ],
                        )
                    else:
                        pat = CFG["dma_build"] if q < NQ - 1 else CFG["dma_last"]
                        if pat[j % len(pat)] == "S":
                            nc.sync.dma_start(dst, stage[:])
                        else:
                            nc.gpsimd.dma_start(dst, stage[:])
                for th in deferred:
                    th()

    nc.compile()
    return nc


def _host_inputs(x, question_emb, interaction_emb, key_memory, value_memory_init):
    """Build the shared constant tensors + per-core shards (all numpy)."""
    import ml_dtypes

    F8 = ml_dtypes.float8_e4m3fn
    x = np.asarray(x).astype(np.int32)
    question_emb = np.asarray(question_emb, dtype=np.float32)
    interaction_emb = np.asarray(interaction_emb, dtype=np.float32)
    key_memory = np.asarray(key_memory, dtype=np.float32)
    value_memory_init = np.asarray(value_memory_init, dtype=np.float32)

    v = np.arange(V, dtype=np.int64)
    qid = (v - 1) % K + 1

    qkcat = np.zeros((EK, V + C), np.float32)
    qkcat[:, :V] = question_emb[qid].T
    qkcat[:, V : V + C] = key_memory.T

    consts = {"qkcat": qkcat, "interemb": interaction_emb}

    in_maps = []
    for core in range(NCORES):
        bs = slice(core * BPC, (core + 1) * BPC)
        xc = x[bs]                                  # [BPC, T]
        # cumulative one-hot counts per batch: ct[v, tau] = #{s<=tau: x[s]=v}
        oh = xc[:, :, None] == np.arange(V)[None, None, :]     # [BPC,T,V]
        ct = np.cumsum(oh, axis=1).transpose(0, 2, 1)          # [BPC,V,T]
        assert ct.max() <= 15, "counts exceed fp8e4m3 exact-integer range"
        cts8 = np.zeros((VS, 2, BPC * T), F8)
        cts8[:, 0, :] = ct[:, 0:VS, :].transpose(1, 0, 2).reshape(VS, BPC * T)
        cts8[0:P1T, 1, :] = (
            ct[:, VS:V, :].transpose(1, 0, 2).reshape(P1T, BPC * T)
        )
        for b in range(BPC):
            cts8[P1T + b, 1, b * T : (b + 1) * T] = 1.0
        initf = value_memory_init[bs].reshape(BPC, F)
        ihi = np.zeros((BPC + 1, F), F8)
        ihi[0:BPC] = initf.astype(F8)
        ilo = np.zeros((BPC + 1, F), F8)
        ilo[0:BPC] = (initf - ihi[0:BPC].astype(np.float32)).astype(F8)
        in_maps.append({**consts, "cts8": cts8, "inithi": ihi, "initlo": ilo})
    return in_maps


def kernel(
    x,
    next_question,
    question_emb,
    interaction_emb,
    key_memory,
    value_memory_init,
):
    from concourse.bass_utils import run_bass_kernel_spmd

    if "nc" not in _CACHE:
        _CACHE["nc"] = _build_program()
    nc = _CACHE["nc"]

    in_maps = _host_inputs(
        x, question_emb, interaction_emb, key_memory, value_memory_init
    )
    res = run_bass_kernel_spmd(nc, in_maps, list(range(NCORES)))
    return _unpack(res.results)


def _unpack(results):
    return np.concatenate(
        [
            np.asarray(r["out"]).astype(np.float32).reshape(BPC, T, C, EI)
            for r in results
        ],
        axis=0,
    )
